# revision 35
# baseline (speedup 1.0000x reference)
"""nn_AugmentShallow (gnn_message_passing) Trainium2 kernel — hybrid.

Per batch b (one NeuronCore each, data-parallel over B=8). The bottleneck
of the pure-gather design is the Pool engine serially generating SWDGE
descriptors at ~2.2ns/desc (98304 descs -> ~218us busy). This kernel
splits tokens between two concurrent resource pools:

  * per-edge path (tokens [0, 512*N_E)): the host pre-gathers raw x rows
    per edge (pure input packing — no FLOPs); the device runs the MLP
    per edge entirely on PE (L1+L2 matmuls) with ACT/DVE relu
    evacuations, DVE tree-add K-sum, and the trans2 tail. Zero
    descriptors.
  * q-gather path (remaining tokens): per-point q table built once
    (strength reduction: q = relu(Wc1 relu(Weff x + beff) + bc1)),
    written token-major to DRAM, then SWDGE dma_gather per edge (Pool
    engine), PE identity K-sum, transpose, trans2 tail.

Both paths write the output channel-major [256, N]; the host transposes.
Engine loads are balanced so Pool (gather path) and PE/ACT/DVE
(per-edge path) saturate together.
"""

import sys

if "/opt/trn_rl_repo" not in sys.path:
    sys.path.insert(0, "/opt/trn_rl_repo")

import numpy as np

B, N, K = 8, 8192, 12
C_IN, C_HID, C_OUT = 3, 128, 256

ECHUNKS = [512] * 9 + [256]          # per-edge chunk tokens
ESTARTS = []
_t = 0
for _e in ECHUNKS:
    ESTARTS.append(_t)
    _t += _e
T1 = _t                              # per-edge tokens [0, T1)
N_E = len(ECHUNKS)
# gather chunks: small leads so the first gather lands before PE runs dry
GCHUNKS = [128] * 4 + [256] * 11
GSTARTS = []
for _g in GCHUNKS:
    GSTARTS.append(_t)
    _t += _g
assert _t == N
N_G = len(GCHUNKS)
N_DUMMY = 4
DUMMY_IDX = 128
N_QUEUES = 4
ECOLS = sum(e * K for e in ECHUNKS)  # total per-edge cols

# evac engine split for per-edge L1/L2: ACT for these h indices, DVE else
ACT_H = (0, 1, 3, 4, 6, 7, 9, 10)   # 8 of 12 on ACT

_CACHE = {}


def _build_program():
    import concourse.bacc as bacc
    import concourse.mybir as mybir
    import concourse.tile as tile

    dt = mybir.dt
    AF = mybir.ActivationFunctionType
    nc = bacc.Bacc("TRN2", target_bir_lowering=False, debug=False, num_devices=8,
                   num_swdge_queues=N_QUEUES)

    idx_slots = N_DUMMY * DUMMY_IDX // 16 + sum(g * K // 16 for g in GCHUNKS)

    xT_d = nc.dram_tensor("xT", [C_IN, N], dt.float16, kind="ExternalInput")
    xg_d = nc.dram_tensor("xg", [C_IN, ECOLS], dt.float16,
                          kind="ExternalInput")
    idx_d = nc.dram_tensor("idx", [128, idx_slots], dt.int16,
                           kind="ExternalInput")
    weffT_d = nc.dram_tensor("weffT", [C_IN, C_HID], dt.float16,
                             kind="ExternalInput")
    beff_d = nc.dram_tensor("beff", [C_HID, 1], dt.float32, kind="ExternalInput")
    wc1T_d = nc.dram_tensor("wc1T", [C_HID, C_HID], dt.float16,
                            kind="ExternalInput")
    bc1_d = nc.dram_tensor("bc1c", [C_HID, 1], dt.float32, kind="ExternalInput")
    w2T_d = nc.dram_tensor("w2T", [C_HID, C_OUT], dt.float16,
                           kind="ExternalInput")
    b2c_d = nc.dram_tensor("b2c", [128, 2], dt.float32, kind="ExternalInput")
    ident_d = nc.dram_tensor("ident", [128, 128], dt.float16,
                             kind="ExternalInput")
    out_d = nc.dram_tensor("out_cm", [C_OUT, N], dt.float16,
                           kind="ExternalOutput")

    with tile.TileContext(nc) as tc:
        with (
            tc.tile_pool(name="const", bufs=1) as cpool,
            tc.tile_pool(name="qb", bufs=3) as qbpool,       # q build p/q slices
            tc.tile_pool(name="qsb", bufs=3) as qspool,      # transposed q rows
            tc.tile_pool(name="xpool", bufs=2) as xpool,     # xcm chunks
            tc.tile_pool(name="pe1", bufs=2) as pe1pool,     # p_sb per chunk
            tc.tile_pool(name="pe2", bufs=2) as pe2pool,     # e_sb per chunk
            tc.tile_pool(name="tree", bufs=2) as treepool,
            tc.tile_pool(name="mpool", bufs=3) as mpool,
            tc.tile_pool(name="gpool", bufs=6) as gpool,     # gather lands
            tc.tile_pool(name="opool", bufs=4) as opool,
            tc.tile_pool(name="qdram", bufs=1, space="DRAM") as dpool,
            tc.tile_pool(name="pp", bufs=4, space="PSUM") as pp,     # 512 f32
            tc.tile_pool(name="po", bufs=2, space="PSUM") as po,     # 512 f32
            tc.tile_pool(name="pt", bufs=2, space="PSUM") as pt,     # 512 f16
        ):
            # ---- persistent SBUF -----------------------------------------
            xT = cpool.tile([C_IN, N], dt.float16)
            weffT = cpool.tile([C_IN, C_HID], dt.float16)
            beff = cpool.tile([C_HID, 1], dt.float32)
            wc1T = cpool.tile([C_HID, C_HID], dt.float16)
            bc1 = cpool.tile([C_HID, 1], dt.float32)
            w2T = cpool.tile([C_HID, C_OUT], dt.float16)
            b2c = cpool.tile([128, 2], dt.float32)
            ident = cpool.tile([128, 128], dt.float16)
            idx = cpool.tile([128, idx_slots], dt.int16)
            q_dram = dpool.tile([N, C_HID], dt.float16)
            scr_dram = dpool.tile([128, C_HID], dt.float16)  # never written

            nc.sync.dma_start(xT[:], xT_d.ap()[:])
            nc.sync.dma_start(weffT[:], weffT_d.ap()[:])
            nc.sync.dma_start(beff[:], beff_d.ap()[:])
            nc.sync.dma_start(wc1T[:], wc1T_d.ap()[:])
            nc.sync.dma_start(bc1[:], bc1_d.ap()[:])
            nc.sync.dma_start(ident[:], ident_d.ap()[:])
            nc.sync.dma_start(idx[:], idx_d.ap()[:])
            nc.sync.dma_start(w2T[:], w2T_d.ap()[:])
            nc.sync.dma_start(b2c[:], b2c_d.ap()[:])

            # ---- warmup dummy gathers (absorb first-gather engine holds) -
            for qn in range(N_DUMMY):
                sg = mpool.tile([128, 1, 128], dt.float16, tag="warm")
                nc.gpsimd.dma_gather(
                    sg[:], scr_dram[:],
                    idx[:, qn * (DUMMY_IDX // 16):(qn + 1) * (DUMMY_IDX // 16)],
                    num_idxs=DUMMY_IDX, num_idxs_reg=DUMMY_IDX,
                    elem_size=C_HID, transpose=False, single_packet=False,
                    queue_num=qn,
                )

            # ---- q table build (two-phase: PE streams back-to-back) ------
            NSL = N // 512
            p_all = cpool.tile([128, N], dt.float16)
            q_sl = [None] * NSL

            for c in range(NSL):
                ps = pp.tile([128, 512], dt.float32, tag="ps512")
                nc.tensor.matmul(ps[:], weffT[:], xT[:, c * 512:(c + 1) * 512],
                                 start=True, stop=True)
                if c % 2 == 0:
                    nc.scalar.activation(p_all[:, c * 512:(c + 1) * 512],
                                         ps[:], AF.Relu, bias=beff[:])
                else:
                    nc.vector.tensor_scalar(
                        p_all[:, c * 512:(c + 1) * 512], ps[:], beff[:], 0.0,
                        op0=mybir.AluOpType.add, op1=mybir.AluOpType.max)

            def qb_l2(c):
                qs = pp.tile([128, 512], dt.float32, tag="ps512")
                nc.tensor.matmul(qs[:], wc1T[:],
                                 p_all[:, c * 512:(c + 1) * 512],
                                 start=True, stop=True)
                q_sl[c] = qbpool.tile([128, 512], dt.float16, tag="q",
                                      name=f"qb_q{c}")
                if c % 2 == 0:
                    nc.vector.tensor_scalar(
                        q_sl[c][:], qs[:], bc1[:], 0.0,
                        op0=mybir.AluOpType.add, op1=mybir.AluOpType.max)
                else:
                    nc.scalar.activation(q_sl[c][:], qs[:], AF.Relu,
                                         bias=bc1[:])

            def qb_tail(c):
                qsb = qspool.tile([128, 512], dt.float16, tag="qsb")
                tq = pt.tile([128, 512], dt.float16, tag="tps",
                             name=f"qtq{c}")
                for s in range(4):
                    nc.tensor.transpose(
                        tq[:, s * 128:(s + 1) * 128],
                        q_sl[c][:, s * 128:(s + 1) * 128], ident[:])
                if c % 2 == 0:
                    nc.vector.tensor_copy(qsb[:], tq[:])
                else:
                    nc.scalar.activation(qsb[:], tq[:], AF.Copy)
                nc.sync.dma_start(
                    q_dram[c * 512:(c + 1) * 512, :]
                    .rearrange("(s p) o -> p s o", p=128),
                    qsb[:].rearrange("p (s o) -> p s o", o=C_HID),
                )

            qb_l2(0)
            qb_l2(1)
            for c in range(2, NSL):
                qb_l2(c)
                qb_tail(c - 2)
            qb_tail(NSL - 2)
            qb_tail(NSL - 1)

            # ---- main loop: interleave gather + per-edge chunks ----------
            g_tiles = [None] * N_G

            gslot0 = []
            _s = N_DUMMY * DUMMY_IDX // 16
            for gch in GCHUNKS:
                gslot0.append(_s)
                _s += gch * K // 16

            def gather_dma(ci):
                ng = GCHUNKS[ci] * K
                g_tiles[ci] = gpool.tile([128, 24, 128], dt.float16,
                                         tag="gf", name=f"g{ci}")
                nc.gpsimd.dma_gather(
                    g_tiles[ci][:, :ng // 128, :], q_dram[:],
                    idx[:, gslot0[ci]:gslot0[ci] + ng // 16],
                    num_idxs=ng, num_idxs_reg=ng,
                    elem_size=C_HID, transpose=False, single_packet=False,
                    queue_num=ci % N_QUEUES,
                )

            # rolling 512-token transpose group: gather chunks accumulate
            # their row-major K-summed m into one psum tile, flushed through
            # a single wide copy + trans2 per 512 tokens
            grp = {"tq": None, "fill": 0, "t0": 0}

            def gather_compute(ci):
                # K-sum on DVE: row-major tree adds over k-major slot blocks
                # (slot j = kb*spk + s: s sub-blocks stay aligned under
                # half-splits 12kb -> 6kb -> 3kb; then two adds over the
                # three remaining kb groups)
                Tg = GCHUNKS[ci]
                spk = Tg // 128
                g = g_tiles[ci]
                with nc.allow_low_precision(reason="fp16 K-sum tree"):
                    t6g = mpool.tile([128, 12, 128], dt.float16, tag="t6g")
                    t6 = t6g[:, :6 * spk, :]
                    nc.vector.tensor_add(
                        t6, g[:, :6 * spk, :], g[:, 6 * spk:12 * spk, :])
                    t3g = mpool.tile([128, 6, 128], dt.float16, tag="t3g")
                    t3 = t3g[:, :3 * spk, :]
                    nc.vector.tensor_add(
                        t3, t6g[:, :3 * spk, :], t6g[:, 3 * spk:6 * spk, :])
                    m2g = mpool.tile([128, 2, 128], dt.float16, tag="m2g")
                    m2 = m2g[:, :spk, :]
                    nc.vector.tensor_add(
                        m2, t3g[:, :spk, :], t3g[:, spk:2 * spk, :])
                    m16 = mpool.tile([128, 256], dt.float16, tag="m16g")
                    nc.vector.tensor_add(
                        m16[:, :Tg].rearrange("p (a t) -> p a t", a=spk),
                        m2, t3g[:, 2 * spk:3 * spk, :])
                if grp["tq"] is None:
                    grp["tq"] = pt.tile([128, 512], dt.float16, tag="tps",
                                        name=f"gtq{ci}")
                    grp["t0"] = GSTARTS[ci]
                fill = grp["fill"]
                for s in range(spk):
                    nc.tensor.transpose(
                        grp["tq"][:, fill + s * 128:fill + (s + 1) * 128],
                        m16[:, s * 128:(s + 1) * 128], ident[:])
                grp["fill"] = fill + Tg
                if grp["fill"] == 512 or ci == N_G - 1:
                    F = grp["fill"]
                    mt = mpool.tile([128, 512], dt.float16, tag="mtg")
                    nc.scalar.activation(mt[:, :F], grp["tq"][:, :F], AF.Copy)
                    trans2_tail(mt[:, :F], grp["t0"], F)
                    grp["tq"] = None
                    grp["fill"] = 0

            def trans2_tail(m16ap, t0, T):
                for h in range(2):
                    ph = po.tile([128, 512], dt.float32, tag="po")
                    nc.tensor.matmul(ph[:, :T],
                                     w2T[:, h * 128:(h + 1) * 128], m16ap,
                                     start=True, stop=True)
                    osb = opool.tile([128, 512], dt.float16, tag="osb")
                    nc.scalar.activation(osb[:, :T], ph[:, :T],
                                         AF.Identity, bias=b2c[:, h:h + 1])
                    nc.sync.dma_start(
                        out_d.ap()[h * 128:(h + 1) * 128, t0:t0 + T],
                        osb[:, :T])

            def edge_chunk(ci):
                t0 = ESTARTS[ci]
                Te = ECHUNKS[ci]
                cols = Te * K
                x0 = ESTARTS[ci] * K
                nblk = cols // 512
                xcm = xpool.tile([C_IN, 6144], dt.float16, tag="xcm",
                                 name=f"xcm{ci}")
                nc.sync.dma_start(xcm[:, :cols], xg_d.ap()[:, x0:x0 + cols])
                p_sb = pe1pool.tile([128, 6144], dt.float16, tag="pe",
                                    name=f"pe{ci}")
                for h in range(nblk):
                    ps = pp.tile([128, 512], dt.float32, tag="ps512")
                    nc.tensor.matmul(ps[:], weffT[:],
                                     xcm[:, h * 512:(h + 1) * 512],
                                     start=True, stop=True)
                    if h % 3 != 2:
                        nc.scalar.activation(p_sb[:, h * 512:(h + 1) * 512],
                                             ps[:], AF.Relu, bias=beff[:])
                    else:
                        nc.vector.tensor_scalar(
                            p_sb[:, h * 512:(h + 1) * 512], ps[:], beff[:],
                            0.0, op0=mybir.AluOpType.add,
                            op1=mybir.AluOpType.max)
                e_sb = pe2pool.tile([128, 6144], dt.float16, tag="ee",
                                    name=f"ee{ci}")
                for h in range(nblk):
                    ps = pp.tile([128, 512], dt.float32, tag="ps512")
                    nc.tensor.matmul(ps[:], wc1T[:],
                                     p_sb[:, h * 512:(h + 1) * 512],
                                     start=True, stop=True)
                    if h % 3 != 2:
                        nc.scalar.activation(e_sb[:, h * 512:(h + 1) * 512],
                                             ps[:], AF.Relu, bias=bc1[:])
                    else:
                        nc.vector.tensor_scalar(
                            e_sb[:, h * 512:(h + 1) * 512], ps[:], bc1[:],
                            0.0, op0=mybir.AluOpType.add,
                            op1=mybir.AluOpType.max)
                # K-sum tree adds on DVE (fp16 2x): 12 -> 6 -> 3 -> 1
                t6 = treepool.tile([128, 6 * 512], dt.float16, tag="t6",
                                   name=f"t6_{ci}")
                with nc.allow_low_precision(reason="fp16 K-sum tree"):
                    nc.vector.tensor_add(
                        t6[:, :6 * Te].rearrange("p (a t) -> p a t", a=6),
                        e_sb[:, :6 * Te].rearrange("p (a t) -> p a t", a=6),
                        e_sb[:, 6 * Te:12 * Te]
                        .rearrange("p (a t) -> p a t", a=6))
                    t3 = treepool.tile([128, 3 * 512], dt.float16, tag="t3",
                                       name=f"t3_{ci}")
                    nc.vector.tensor_add(
                        t3[:, :3 * Te].rearrange("p (a t) -> p a t", a=3),
                        t6[:, :3 * Te].rearrange("p (a t) -> p a t", a=3),
                        t6[:, 3 * Te:6 * Te].rearrange("p (a t) -> p a t", a=3))
                    t1 = mpool.tile([128, 512], dt.float16, tag="t1",
                                    name=f"t1_{ci}")
                    nc.vector.tensor_add(t1[:, :Te], t3[:, :Te],
                                         t3[:, Te:2 * Te])
                    m16 = mpool.tile([128, 512], dt.float16, tag="m16e",
                                     name=f"m16e{ci}")
                    nc.vector.tensor_add(m16[:, :Te], t1[:, :Te],
                                         t3[:, 2 * Te:3 * Te])
                trans2_tail(m16[:, :Te], t0, Te)

            # gather DMAs pipelined DEPTH ahead of their compute so no
            # engine queue head ever waits on un-landed gather data
            DEPTH = 5
            for i in range(max(N_G + DEPTH, N_E)):
                if i < N_G:
                    gather_dma(i)
                if i < N_E:
                    edge_chunk(i)
                if DEPTH <= i < N_G + DEPTH:
                    gather_compute(i - DEPTH)

    nc.compile()
    return nc


def _get_program():
    if "nc" not in _CACHE:
        _CACHE["nc"] = _build_program()
    return _CACHE["nc"]


def _host_prep(x, knn_idx, W1, b1, Wc0, bc0, Wc1, bc1, W2, b2):
    f64 = np.float64
    weff = (Wc0.astype(f64) @ W1.astype(f64))                    # [128, 3]
    beff = (Wc0.astype(f64) @ b1.astype(f64) + bc0.astype(f64))  # [128]
    w2s = W2.astype(f64) / K                                     # fold 1/K

    weffT = np.ascontiguousarray(weff.T.astype(np.float16))
    beff_c = np.ascontiguousarray(beff.astype(np.float32)[:, None])
    wc1T = np.ascontiguousarray(Wc1.T.astype(np.float16))
    bc1_c = np.ascontiguousarray(bc1.astype(np.float32)[:, None])
    w2T = np.ascontiguousarray(w2s.T.astype(np.float16))
    b2c = np.ascontiguousarray(
        b2.astype(np.float32).reshape(2, 128).T)                 # [128, 2]
    ident = np.eye(128, dtype=np.float16)

    in_maps = []
    for bi in range(B):
        xb = x[bi]                                               # [N, 3]
        xT = np.ascontiguousarray(xb.T.astype(np.float16))
        kb = knn_idx[bi]

        # per-edge path: host gathers raw x rows, k-major per chunk
        xg_cols = []
        for ci in range(N_E):
            t0 = ESTARTS[ci]
            Te = ECHUNKS[ci]
            flat = np.ascontiguousarray(kb[t0:t0 + Te, :].T).reshape(-1)
            xg_cols.append(xb[flat].T)                           # [3, Te*K]
        xg = np.ascontiguousarray(
            np.concatenate(xg_cols, axis=1).astype(np.float16))

        # gather path: k-major idx per chunk, 16-wrapped, core-replicated
        cols = [np.zeros((128, N_DUMMY * DUMMY_IDX // 16), dtype=np.int16)]
        for ci in range(N_G):
            t0 = GSTARTS[ci]
            Tg = GCHUNKS[ci]
            flat = np.ascontiguousarray(
                kb[t0:t0 + Tg, :].T).reshape(-1).astype(np.int16)
            wrapped = flat.reshape(Tg * K // 16, 16).T
            cols.append(np.tile(wrapped, (8, 1)))
        idx = np.ascontiguousarray(np.concatenate(cols, axis=1))

        in_maps.append({
            "xT": xT, "xg": xg, "idx": idx, "weffT": weffT, "beff": beff_c,
            "wc1T": wc1T, "bc1c": bc1_c, "w2T": w2T, "b2c": b2c,
            "ident": ident,
        })
    return in_maps


def kernel(x, knn_idx, W1, b1, Wc0, bc0, Wc1, bc1, W2, b2):
    x = np.asarray(x)
    knn_idx = np.asarray(knn_idx)
    args = [np.asarray(a) for a in (W1, b1, Wc0, bc0, Wc1, bc1, W2, b2)]
    in_maps = _host_prep(x, knn_idx, *args)
    nc = _get_program()
    from concourse import bass_utils
    res = bass_utils.run_bass_kernel_spmd(nc, in_maps, core_ids=list(range(B)))
    return np.stack(
        [np.ascontiguousarray(res.results[i]["out_cm"].T.astype(np.float32)) for i in range(B)],
        axis=0)


# revision 39
# speedup vs baseline: 1.0077x; 1.0077x over previous
"""nn_AugmentShallow (gnn_message_passing) Trainium2 kernel — hybrid.

Per batch b (one NeuronCore each, data-parallel over B=8). The bottleneck
of the pure-gather design is the Pool engine serially generating SWDGE
descriptors at ~2.2ns/desc (98304 descs -> ~218us busy). This kernel
splits tokens between two concurrent resource pools:

  * per-edge path (tokens [0, 512*N_E)): the host pre-gathers raw x rows
    per edge (pure input packing — no FLOPs); the device runs the MLP
    per edge entirely on PE (L1+L2 matmuls) with ACT/DVE relu
    evacuations, DVE tree-add K-sum, and the trans2 tail. Zero
    descriptors.
  * q-gather path (remaining tokens): per-point q table built once
    (strength reduction: q = relu(Wc1 relu(Weff x + beff) + bc1)),
    written token-major to DRAM, then SWDGE dma_gather per edge (Pool
    engine), PE identity K-sum, transpose, trans2 tail.

Both paths write the output channel-major [256, N]; the host transposes.
Engine loads are balanced so Pool (gather path) and PE/ACT/DVE
(per-edge path) saturate together.
"""

import sys

if "/opt/trn_rl_repo" not in sys.path:
    sys.path.insert(0, "/opt/trn_rl_repo")

import numpy as np

B, N, K = 8, 8192, 12
C_IN, C_HID, C_OUT = 3, 128, 256

ECHUNKS = [512] * 9 + [256]          # per-edge chunk tokens
ESTARTS = []
_t = 0
for _e in ECHUNKS:
    ESTARTS.append(_t)
    _t += _e
T1 = _t                              # per-edge tokens [0, T1)
N_E = len(ECHUNKS)
# gather chunks: small leads so the first gather lands before PE runs dry
GCHUNKS = [128] * 4 + [256] * 11
GSTARTS = []
for _g in GCHUNKS:
    GSTARTS.append(_t)
    _t += _g
assert _t == N
N_G = len(GCHUNKS)
N_DUMMY = 4
DUMMY_IDX = 128
N_QUEUES = 4
ECOLS = sum(e * K for e in ECHUNKS)  # total per-edge cols

# evac engine split for per-edge L1/L2: ACT for these h indices, DVE else
ACT_H = (0, 1, 3, 4, 6, 7, 9, 10)   # 8 of 12 on ACT

_CACHE = {}


def _build_program():
    import concourse.bacc as bacc
    import concourse.mybir as mybir
    import concourse.tile as tile

    dt = mybir.dt
    AF = mybir.ActivationFunctionType
    nc = bacc.Bacc("TRN2", target_bir_lowering=False, debug=False, num_devices=8,
                   num_swdge_queues=N_QUEUES)

    idx_slots = N_DUMMY * DUMMY_IDX // 16 + sum(g * K // 16 for g in GCHUNKS)

    xT_d = nc.dram_tensor("xT", [C_IN, N], dt.float16, kind="ExternalInput")
    xg_d = nc.dram_tensor("xg", [C_IN, ECOLS], dt.float16,
                          kind="ExternalInput")
    idx_d = nc.dram_tensor("idx", [128, idx_slots], dt.int16,
                           kind="ExternalInput")
    weffT_d = nc.dram_tensor("weffT", [C_IN, C_HID], dt.float16,
                             kind="ExternalInput")
    beff_d = nc.dram_tensor("beff", [C_HID, 1], dt.float32, kind="ExternalInput")
    wc1T_d = nc.dram_tensor("wc1T", [C_HID, C_HID], dt.float16,
                            kind="ExternalInput")
    bc1_d = nc.dram_tensor("bc1c", [C_HID, 1], dt.float32, kind="ExternalInput")
    w2T_d = nc.dram_tensor("w2T", [C_HID, C_OUT], dt.float16,
                           kind="ExternalInput")
    b2c_d = nc.dram_tensor("b2c", [128, 2], dt.float32, kind="ExternalInput")
    ident_d = nc.dram_tensor("ident", [128, 128], dt.float16,
                             kind="ExternalInput")
    out_d = nc.dram_tensor("out_cm", [C_OUT, N], dt.float16,
                           kind="ExternalOutput")

    with tile.TileContext(nc) as tc:
        with (
            tc.tile_pool(name="const", bufs=1) as cpool,
            tc.tile_pool(name="qb", bufs=3) as qbpool,       # q build p/q slices
            tc.tile_pool(name="qsb", bufs=3) as qspool,      # transposed q rows
            tc.tile_pool(name="xpool", bufs=2) as xpool,     # xcm chunks
            tc.tile_pool(name="pe1", bufs=2) as pe1pool,     # p_sb per chunk
            tc.tile_pool(name="pe2", bufs=2) as pe2pool,     # e_sb per chunk
            tc.tile_pool(name="tree", bufs=2) as treepool,
            tc.tile_pool(name="mpool", bufs=3) as mpool,
            tc.tile_pool(name="gpool", bufs=6) as gpool,     # gather lands
            tc.tile_pool(name="opool", bufs=4) as opool,
            tc.tile_pool(name="qdram", bufs=1, space="DRAM") as dpool,
            tc.tile_pool(name="pp", bufs=4, space="PSUM") as pp,     # 512 f32
            tc.tile_pool(name="po", bufs=2, space="PSUM") as po,     # 512 f32
            tc.tile_pool(name="pt", bufs=2, space="PSUM") as pt,     # 512 f16
        ):
            # ---- persistent SBUF -----------------------------------------
            xT = cpool.tile([C_IN, N], dt.float16)
            weffT = cpool.tile([C_IN, C_HID], dt.float16)
            beff = cpool.tile([C_HID, 1], dt.float32)
            wc1T = cpool.tile([C_HID, C_HID], dt.float16)
            bc1 = cpool.tile([C_HID, 1], dt.float32)
            w2T = cpool.tile([C_HID, C_OUT], dt.float16)
            b2c = cpool.tile([128, 2], dt.float32)
            ident = cpool.tile([128, 128], dt.float16)
            idx = cpool.tile([128, idx_slots], dt.int16)
            q_dram = dpool.tile([N, C_HID], dt.float16)
            scr_dram = dpool.tile([128, C_HID], dt.float16)  # never written

            nc.sync.dma_start(xT[:], xT_d.ap()[:])
            nc.sync.dma_start(weffT[:], weffT_d.ap()[:])
            nc.sync.dma_start(beff[:], beff_d.ap()[:])
            nc.sync.dma_start(wc1T[:], wc1T_d.ap()[:])
            nc.sync.dma_start(bc1[:], bc1_d.ap()[:])
            nc.sync.dma_start(ident[:], ident_d.ap()[:])
            nc.sync.dma_start(idx[:], idx_d.ap()[:])
            nc.sync.dma_start(w2T[:], w2T_d.ap()[:])
            nc.sync.dma_start(b2c[:], b2c_d.ap()[:])

            # ---- warmup dummy gathers (absorb first-gather engine holds) -
            for qn in range(N_DUMMY):
                sg = mpool.tile([128, 1, 128], dt.float16, tag="warm")
                nc.gpsimd.dma_gather(
                    sg[:], scr_dram[:],
                    idx[:, qn * (DUMMY_IDX // 16):(qn + 1) * (DUMMY_IDX // 16)],
                    num_idxs=DUMMY_IDX, num_idxs_reg=DUMMY_IDX,
                    elem_size=C_HID, transpose=False, single_packet=False,
                    queue_num=qn,
                )

            # ---- q table build (two-phase: PE streams back-to-back) ------
            NSL = N // 512
            p_all = cpool.tile([128, N], dt.float16)
            q_sl = [None] * NSL

            for c in range(NSL):
                ps = pp.tile([128, 512], dt.float32, tag="ps512")
                nc.tensor.matmul(ps[:], weffT[:], xT[:, c * 512:(c + 1) * 512],
                                 start=True, stop=True)
                if c % 2 == 0:
                    nc.scalar.activation(p_all[:, c * 512:(c + 1) * 512],
                                         ps[:], AF.Relu, bias=beff[:])
                else:
                    nc.vector.tensor_scalar(
                        p_all[:, c * 512:(c + 1) * 512], ps[:], beff[:], 0.0,
                        op0=mybir.AluOpType.add, op1=mybir.AluOpType.max)

            def qb_l2(c):
                qs = pp.tile([128, 512], dt.float32, tag="ps512")
                nc.tensor.matmul(qs[:], wc1T[:],
                                 p_all[:, c * 512:(c + 1) * 512],
                                 start=True, stop=True)
                q_sl[c] = qbpool.tile([128, 512], dt.float16, tag="q",
                                      name=f"qb_q{c}")
                if c % 2 == 0:
                    nc.vector.tensor_scalar(
                        q_sl[c][:], qs[:], bc1[:], 0.0,
                        op0=mybir.AluOpType.add, op1=mybir.AluOpType.max)
                else:
                    nc.scalar.activation(q_sl[c][:], qs[:], AF.Relu,
                                         bias=bc1[:])

            def qb_tail(c):
                qsb = qspool.tile([128, 512], dt.float16, tag="qsb")
                tq = pt.tile([128, 512], dt.float16, tag="tps",
                             name=f"qtq{c}")
                for s in range(4):
                    nc.tensor.transpose(
                        tq[:, s * 128:(s + 1) * 128],
                        q_sl[c][:, s * 128:(s + 1) * 128], ident[:])
                if c % 2 == 0:
                    nc.vector.tensor_copy(qsb[:], tq[:])
                else:
                    nc.scalar.activation(qsb[:], tq[:], AF.Copy)
                nc.sync.dma_start(
                    q_dram[c * 512:(c + 1) * 512, :]
                    .rearrange("(s p) o -> p s o", p=128),
                    qsb[:].rearrange("p (s o) -> p s o", o=C_HID),
                )

            qb_l2(0)
            qb_l2(1)
            for c in range(2, NSL):
                qb_l2(c)
                qb_tail(c - 2)
            qb_tail(NSL - 2)
            qb_tail(NSL - 1)

            # ---- main loop: interleave gather + per-edge chunks ----------
            g_tiles = [None] * N_G

            gslot0 = []
            _s = N_DUMMY * DUMMY_IDX // 16
            for gch in GCHUNKS:
                gslot0.append(_s)
                _s += gch * K // 16

            def gather_dma(ci):
                ng = GCHUNKS[ci] * K
                g_tiles[ci] = gpool.tile([128, 24, 128], dt.float16,
                                         tag="gf", name=f"g{ci}")
                nc.gpsimd.dma_gather(
                    g_tiles[ci][:, :ng // 128, :], q_dram[:],
                    idx[:, gslot0[ci]:gslot0[ci] + ng // 16],
                    num_idxs=ng, num_idxs_reg=ng,
                    elem_size=C_HID, transpose=False, single_packet=False,
                    queue_num=ci % N_QUEUES,
                )

            # rolling 512-token transpose group: gather chunks accumulate
            # their row-major K-summed m into one psum tile, flushed through
            # a single wide copy + trans2 per 512 tokens
            grp = {"tq": None, "fill": 0, "t0": 0}

            def gather_compute(ci):
                # K-sum on DVE: row-major tree adds over k-major slot blocks
                # (slot j = kb*spk + s: s sub-blocks stay aligned under
                # half-splits 12kb -> 6kb -> 3kb; then two adds over the
                # three remaining kb groups)
                Tg = GCHUNKS[ci]
                spk = Tg // 128
                g = g_tiles[ci]
                with nc.allow_low_precision(reason="fp16 K-sum tree"):
                    t6g = mpool.tile([128, 12, 128], dt.float16, tag="t6g")
                    t6 = t6g[:, :6 * spk, :]
                    nc.vector.tensor_add(
                        t6, g[:, :6 * spk, :], g[:, 6 * spk:12 * spk, :])
                    t3g = mpool.tile([128, 6, 128], dt.float16, tag="t3g")
                    t3 = t3g[:, :3 * spk, :]
                    nc.vector.tensor_add(
                        t3, t6g[:, :3 * spk, :], t6g[:, 3 * spk:6 * spk, :])
                    m2g = mpool.tile([128, 2, 128], dt.float16, tag="m2g")
                    m2 = m2g[:, :spk, :]
                    nc.vector.tensor_add(
                        m2, t3g[:, :spk, :], t3g[:, spk:2 * spk, :])
                    m16 = mpool.tile([128, 256], dt.float16, tag="m16g")
                    nc.vector.tensor_add(
                        m16[:, :Tg].rearrange("p (a t) -> p a t", a=spk),
                        m2, t3g[:, 2 * spk:3 * spk, :])
                if grp["tq"] is None:
                    grp["tq"] = pt.tile([128, 512], dt.float16, tag="tps",
                                        name=f"gtq{ci}")
                    grp["t0"] = GSTARTS[ci]
                fill = grp["fill"]
                for s in range(spk):
                    nc.tensor.transpose(
                        grp["tq"][:, fill + s * 128:fill + (s + 1) * 128],
                        m16[:, s * 128:(s + 1) * 128], ident[:])
                grp["fill"] = fill + Tg
                if grp["fill"] == 512 or ci == N_G - 1:
                    F = grp["fill"]
                    mt = mpool.tile([128, 512], dt.float16, tag="mtg")
                    nc.scalar.activation(mt[:, :F], grp["tq"][:, :F], AF.Copy)
                    trans2_tail(mt[:, :F], grp["t0"], F)
                    grp["tq"] = None
                    grp["fill"] = 0

            def trans2_tail(m16ap, t0, T):
                for h in range(2):
                    ph = po.tile([128, 512], dt.float32, tag="po")
                    nc.tensor.matmul(ph[:, :T],
                                     w2T[:, h * 128:(h + 1) * 128], m16ap,
                                     start=True, stop=True)
                    osb = opool.tile([128, 512], dt.float16, tag="osb")
                    if h == 0:
                        nc.scalar.activation(osb[:, :T], ph[:, :T],
                                             AF.Identity, bias=b2c[:, 0:1])
                    else:
                        nc.vector.tensor_scalar(
                            osb[:, :T], ph[:, :T], b2c[:, 1:2], None,
                            op0=mybir.AluOpType.add)
                    nc.sync.dma_start(
                        out_d.ap()[h * 128:(h + 1) * 128, t0:t0 + T],
                        osb[:, :T])

            def edge_chunk(ci):
                t0 = ESTARTS[ci]
                Te = ECHUNKS[ci]
                cols = Te * K
                x0 = ESTARTS[ci] * K
                nblk = cols // 512
                xcm = xpool.tile([C_IN, 6144], dt.float16, tag="xcm",
                                 name=f"xcm{ci}")
                nc.sync.dma_start(xcm[:, :cols], xg_d.ap()[:, x0:x0 + cols])
                p_sb = pe1pool.tile([128, 6144], dt.float16, tag="pe",
                                    name=f"pe{ci}")
                for h in range(nblk):
                    ps = pp.tile([128, 512], dt.float32, tag="ps512")
                    nc.tensor.matmul(ps[:], weffT[:],
                                     xcm[:, h * 512:(h + 1) * 512],
                                     start=True, stop=True)
                    if h % 3 != 2:
                        nc.scalar.activation(p_sb[:, h * 512:(h + 1) * 512],
                                             ps[:], AF.Relu, bias=beff[:])
                    else:
                        nc.vector.tensor_scalar(
                            p_sb[:, h * 512:(h + 1) * 512], ps[:], beff[:],
                            0.0, op0=mybir.AluOpType.add,
                            op1=mybir.AluOpType.max)
                e_sb = pe2pool.tile([128, 6144], dt.float16, tag="ee",
                                    name=f"ee{ci}")
                for h in range(nblk):
                    ps = pp.tile([128, 512], dt.float32, tag="ps512")
                    nc.tensor.matmul(ps[:], wc1T[:],
                                     p_sb[:, h * 512:(h + 1) * 512],
                                     start=True, stop=True)
                    if h % 3 != 2:
                        nc.scalar.activation(e_sb[:, h * 512:(h + 1) * 512],
                                             ps[:], AF.Relu, bias=bc1[:])
                    else:
                        nc.vector.tensor_scalar(
                            e_sb[:, h * 512:(h + 1) * 512], ps[:], bc1[:],
                            0.0, op0=mybir.AluOpType.add,
                            op1=mybir.AluOpType.max)
                # K-sum tree adds on DVE (fp16 2x): 12 -> 6 -> 3 -> 1
                t6 = treepool.tile([128, 6 * 512], dt.float16, tag="t6",
                                   name=f"t6_{ci}")
                with nc.allow_low_precision(reason="fp16 K-sum tree"):
                    nc.vector.tensor_add(
                        t6[:, :6 * Te].rearrange("p (a t) -> p a t", a=6),
                        e_sb[:, :6 * Te].rearrange("p (a t) -> p a t", a=6),
                        e_sb[:, 6 * Te:12 * Te]
                        .rearrange("p (a t) -> p a t", a=6))
                    t3 = treepool.tile([128, 3 * 512], dt.float16, tag="t3",
                                       name=f"t3_{ci}")
                    nc.vector.tensor_add(
                        t3[:, :3 * Te].rearrange("p (a t) -> p a t", a=3),
                        t6[:, :3 * Te].rearrange("p (a t) -> p a t", a=3),
                        t6[:, 3 * Te:6 * Te].rearrange("p (a t) -> p a t", a=3))
                    t1 = mpool.tile([128, 512], dt.float16, tag="t1",
                                    name=f"t1_{ci}")
                    nc.vector.tensor_add(t1[:, :Te], t3[:, :Te],
                                         t3[:, Te:2 * Te])
                    m16 = mpool.tile([128, 512], dt.float16, tag="m16e",
                                     name=f"m16e{ci}")
                    nc.vector.tensor_add(m16[:, :Te], t1[:, :Te],
                                         t3[:, 2 * Te:3 * Te])
                trans2_tail(m16[:, :Te], t0, Te)

            # gather DMAs pipelined DEPTH ahead of their compute so no
            # engine queue head ever waits on un-landed gather data
            DEPTH = 4
            for i in range(max(N_G + DEPTH, N_E)):
                if i < N_G:
                    gather_dma(i)
                if i < N_E:
                    edge_chunk(i)
                if DEPTH <= i < N_G + DEPTH:
                    gather_compute(i - DEPTH)

    nc.compile()
    return nc


def _get_program():
    if "nc" not in _CACHE:
        _CACHE["nc"] = _build_program()
    return _CACHE["nc"]


def _host_prep(x, knn_idx, W1, b1, Wc0, bc0, Wc1, bc1, W2, b2):
    f64 = np.float64
    weff = (Wc0.astype(f64) @ W1.astype(f64))                    # [128, 3]
    beff = (Wc0.astype(f64) @ b1.astype(f64) + bc0.astype(f64))  # [128]
    w2s = W2.astype(f64) / K                                     # fold 1/K

    weffT = np.ascontiguousarray(weff.T.astype(np.float16))
    beff_c = np.ascontiguousarray(beff.astype(np.float32)[:, None])
    wc1T = np.ascontiguousarray(Wc1.T.astype(np.float16))
    bc1_c = np.ascontiguousarray(bc1.astype(np.float32)[:, None])
    w2T = np.ascontiguousarray(w2s.T.astype(np.float16))
    b2c = np.ascontiguousarray(
        b2.astype(np.float32).reshape(2, 128).T)                 # [128, 2]
    ident = np.eye(128, dtype=np.float16)

    in_maps = []
    for bi in range(B):
        xb = x[bi]                                               # [N, 3]
        xT = np.ascontiguousarray(xb.T.astype(np.float16))
        kb = knn_idx[bi]

        # per-edge path: host gathers raw x rows, k-major per chunk
        xg_cols = []
        for ci in range(N_E):
            t0 = ESTARTS[ci]
            Te = ECHUNKS[ci]
            flat = np.ascontiguousarray(kb[t0:t0 + Te, :].T).reshape(-1)
            xg_cols.append(xb[flat].T)                           # [3, Te*K]
        xg = np.ascontiguousarray(
            np.concatenate(xg_cols, axis=1).astype(np.float16))

        # gather path: k-major idx per chunk, 16-wrapped, core-replicated
        cols = [np.zeros((128, N_DUMMY * DUMMY_IDX // 16), dtype=np.int16)]
        for ci in range(N_G):
            t0 = GSTARTS[ci]
            Tg = GCHUNKS[ci]
            flat = np.ascontiguousarray(
                kb[t0:t0 + Tg, :].T).reshape(-1).astype(np.int16)
            wrapped = flat.reshape(Tg * K // 16, 16).T
            cols.append(np.tile(wrapped, (8, 1)))
        idx = np.ascontiguousarray(np.concatenate(cols, axis=1))

        in_maps.append({
            "xT": xT, "xg": xg, "idx": idx, "weffT": weffT, "beff": beff_c,
            "wc1T": wc1T, "bc1c": bc1_c, "w2T": w2T, "b2c": b2c,
            "ident": ident,
        })
    return in_maps


def kernel(x, knn_idx, W1, b1, Wc0, bc0, Wc1, bc1, W2, b2):
    x = np.asarray(x)
    knn_idx = np.asarray(knn_idx)
    args = [np.asarray(a) for a in (W1, b1, Wc0, bc0, Wc1, bc1, W2, b2)]
    in_maps = _host_prep(x, knn_idx, *args)
    nc = _get_program()
    from concourse import bass_utils
    res = bass_utils.run_bass_kernel_spmd(nc, in_maps, core_ids=list(range(B)))
    return np.stack(
        [np.ascontiguousarray(res.results[i]["out_cm"].T.astype(np.float32)) for i in range(B)],
        axis=0)
